# revision 26
# baseline (speedup 1.0000x reference)
"""Bahdanau-attention alignment scores on 8 TRN2 NeuronCores.

scores[b, n] = v_a . tanh(W_a @ s_prev[b] + U_a @ h_j[b, n])

Strategy: data-parallel over batch (8 batches/core, no collectives).
Per core the dominant cost is the (8192 x 2048) @ (2048 x 1024) GEMM
(h_j shard vs U_a^T), run at full PE rate (1 cycle/row) in GEMM_DT —
bf16 by default, or float32r (TF32, near-fp32 accuracy, ~5% slower)
via the one-line switch below — with fp32 PSUM accumulation. H lives
on the PSUM partition axis, so the W_a@s bias (computed on host: 0.05%
of total FLOPs) is a per-partition scalar fused into the ScalarE tanh.
The v_a-weighted reduction over H runs as a per-partition DVE
multiply-accumulate, finished by one 128->1 partition-reduce matmul
per m-tile (the last m-tile instead reduces incrementally per h-chunk
to shorten the kernel tail).

Head latency is hidden by chunking the U_a^T and first-h_j DMA loads
and by warming the PE clock gate with junk matmuls during the preload.
Inputs are rounded to the GEMM dtype and laid out host-side so every
device DMA is a plain [128, free]-contiguous load.
"""
import os
import sys

sys.path.insert(0, "/opt/trn_rl_repo")

import numpy as np

# ---------------------------------------------------------------------------
# Make NTFF profiling under axon work even if the image's antenv lacks the
# hook-registry module (concourse imports it when trace=True).
_HOOK_SRC = '''\
_HOOK = None


def set_axon_ntff_profile_hook(hook) -> None:
    global _HOOK
    _HOOK = hook


def get_axon_ntff_profile_hook():
    return _HOOK
'''


def _ensure_axon_hooks() -> None:
    try:
        import antenv

        path = os.path.join(list(antenv.__path__)[0], "axon_hooks.py")
        if not os.path.exists(path):
            with open(path, "w") as f:
                f.write(_HOOK_SRC)
    except Exception:
        pass


_ensure_axon_hooks()

import concourse.bass as bass  # noqa: E402
import concourse.tile as tile  # noqa: E402
from concourse import bacc, mybir  # noqa: E402
from concourse.bass_utils import run_bass_kernel_spmd  # noqa: E402

# Problem shapes (hardcoded per spec).
B, NSEQ, C, H = 64, 1024, 2000, 1000
NCORES = 8
BL = B // NCORES          # batches per core
CP, HP = 2048, 1024       # padded contraction / hidden dims
KC, KH = CP // 128, HP // 128   # 16, 8 chunks
ML = BL * NSEQ            # rows per core (8192)
MT = 512                  # m-tile (columns per PSUM bank)
T = ML // MT              # m-tiles per core (16)

F32 = mybir.dt.float32
F32R = mybir.dt.float32r
BF16 = mybir.dt.bfloat16

# GEMM operand dtype: F32R (TF32 precision, ~231 ns/MM) or BF16 (~216 ns/MM).
GEMM_DT = BF16


def _tf32(x: np.ndarray) -> np.ndarray:
    """Round fp32 to the TF32 (fp32r) grid, round-to-nearest."""
    u = np.ascontiguousarray(x, dtype=np.float32).view(np.uint32)
    return ((u + 0x1000) & 0xFFFFE000).view(np.float32)


def _build():
    nc = bacc.Bacc(None, target_bir_lowering=False)
    hj_d = nc.declare_dram_parameter("hj", [T, 128, KC * MT], GEMM_DT, isOutput=False)
    u_d = nc.declare_dram_parameter("u", [KC, 128, HP], GEMM_DT, isOutput=False)
    ws_d = nc.declare_dram_parameter("ws", [128, KH * BL], F32, isOutput=False)
    v_d = nc.declare_dram_parameter("v", [128, KH], F32, isOutput=False)
    vg_d = nc.declare_dram_parameter("vg", [128, KH * 128], GEMM_DT, isOutput=False)
    one_d = nc.declare_dram_parameter("one", [128, 128], F32R, isOutput=False)
    out_d = nc.declare_dram_parameter("out", [BL, NSEQ], F32, isOutput=True)

    with tile.TileContext(nc) as tc:
        with (
            tc.tile_pool(name="const", bufs=1) as cpool,
            tc.tile_pool(name="upool", bufs=1) as upool,
            tc.tile_pool(name="hj0p", bufs=1) as hj0p,
            tc.tile_pool(name="hjp", bufs=3) as hjp,
            tc.tile_pool(name="enp", bufs=4) as enp,
            tc.tile_pool(name="tmp_p", bufs=3) as tmp_p,
            tc.tile_pool(name="accp", bufs=3) as accp,
            tc.tile_pool(name="obp", bufs=2) as obp,
            tc.tile_pool(name="pse_p", bufs=6, space="PSUM") as pse_p,
            tc.tile_pool(name="psf_p", bufs=2, space="PSUM") as psf_p,
        ):
            ws_sb = cpool.tile([128, KH * BL], F32)
            v_sb = cpool.tile([128, KH], F32)
            vg_sb = cpool.tile([128, KH * 128], GEMM_DT)
            warm_sb = cpool.tile([128, 512], BF16)
            nc.gpsimd.memset(warm_sb[:], 1.0)
            one_sb = cpool.tile([128, 128], F32R)

            # U chunks + first-tile h_j chunks, interleaved so the k=0 work
            # is ready almost immediately. Late-consumed constants (ws at
            # the first tanh, one at the first flush, vg only at the last
            # tile) queue behind the first compute-critical chunks.
            u_sb = []
            hj0 = []
            for k in range(KC):
                ut = upool.tile([128, HP], GEMM_DT, name=f"u{k}")
                nc.sync.dma_start(ut[:], u_d[k])
                u_sb.append(ut)
                h0 = hj0p.tile([128, MT], GEMM_DT, name=f"hj0_{k}")
                nc.sync.dma_start(h0[:], hj_d[0][:, k * MT : (k + 1) * MT])
                hj0.append(h0)
                if k == 1:
                    nc.sync.dma_start(ws_sb[:], ws_d[:])
                    nc.sync.dma_start(v_sb[:], v_d[:])
                elif k == 3:
                    nc.sync.dma_start(one_sb[:], one_d[:])
                elif k == KC - 1:
                    nc.sync.dma_start(vg_sb[:], vg_d[:])

            # Warm the PE clock gate (HAM) during the input preload so the
            # first real matmuls run at 2.4 GHz.
            psj = psf_p.tile([128, MT], F32, name="psf")
            for _ in range(16):
                nc.tensor.matmul(
                    psj[:], warm_sb[:, 0:128], warm_sb[:], start=True, stop=True
                )

            def dve_accum(acc, en, hk):
                if hk == 0:
                    nc.vector.tensor_scalar_mul(acc[:], en[:], v_sb[:, 0:1])
                else:
                    tmp = tmp_p.tile([128, MT], F32R, name="tmp")
                    nc.vector.tensor_scalar_mul(tmp[:], en[:], v_sb[:, hk : hk + 1])
                    nc.vector.tensor_add(acc[:], acc[:], tmp[:])

            def tanh_into(en, pse, hk, b):
                nc.scalar.activation(
                    en[:],
                    pse[:],
                    mybir.ActivationFunctionType.Tanh,
                    bias=ws_sb[:, hk * BL + b : hk * BL + b + 1],
                )

            pending = None

            def flush_pending():
                nonlocal pending
                if pending is None:
                    return
                f_acc, f_b, f_j0 = pending
                pending = None
                psf = psf_p.tile([128, MT], F32, name="psf")
                nc.tensor.matmul(
                    psf[:], one_sb[:], f_acc[:], start=True, stop=True
                )
                ob = obp.tile([1, MT], F32, name="ob")
                nc.vector.tensor_copy(ob[:], psf[0:1, :])
                nc.sync.dma_start(out_d[f_b : f_b + 1, f_j0 : f_j0 + MT], ob[:])

            for t in range(T):
                b = t // (NSEQ // MT)
                last = t == T - 1
                if t == 0:
                    mov = [hj0[k][:] for k in range(KC)]
                else:
                    hjt = hjp.tile([128, KC * MT], GEMM_DT, name="hjt")
                    nc.sync.dma_start(hjt[:], hj_d[t])
                    mov = [hjt[:, k * MT : (k + 1) * MT] for k in range(KC)]
                if last:
                    psf = psf_p.tile([128, MT], F32, name="psf")
                    prev_red = None
                else:
                    acc = accp.tile([128, MT], F32R, name="acc")
                for hk in range(KH):
                    pse = pse_p.tile([128, MT], F32, name="pse")
                    for k in range(KC):
                        nc.tensor.matmul(
                            pse[:],
                            u_sb[k][:, hk * 128 : (hk + 1) * 128],
                            mov[k],
                            start=(k == 0),
                            stop=(k == KC - 1),
                        )
                    if hk == 0:
                        flush_pending()
                    if last:
                        # incremental per-chunk reduce, deferred one group so
                        # the in-order PE never waits on the producing tanh
                        if prev_red is not None:
                            ph, pen = prev_red
                            nc.tensor.matmul(
                                psf[:],
                                vg_sb[:, ph * 128 : (ph + 1) * 128],
                                pen[:],
                                start=(ph == 0),
                                stop=False,
                            )
                        en = enp.tile([128, MT], GEMM_DT, name="eng")
                        tanh_into(en, pse, hk, b)
                        prev_red = (hk, en)
                    else:
                        en = enp.tile([128, MT], F32, name="en")
                        tanh_into(en, pse, hk, b)
                        dve_accum(acc, en, hk)
                j0 = (t % (NSEQ // MT)) * MT
                if last:
                    ph, pen = prev_red
                    nc.tensor.matmul(
                        psf[:],
                        vg_sb[:, ph * 128 : (ph + 1) * 128],
                        pen[:],
                        start=False,
                        stop=True,
                    )
                    ob = obp.tile([1, MT], F32, name="ob")
                    nc.vector.tensor_copy(ob[:], psf[0:1, :])
                    nc.sync.dma_start(out_d[b : b + 1, j0 : j0 + MT], ob[:])
                else:
                    pending = (acc, b, j0)
    nc.compile()
    return nc


_NC = None


def _get_nc():
    global _NC
    if _NC is None:
        _NC = _build()
    return _NC


def _prep_inputs(s_prev, h_j, W_a, U_a, v_a):
    """Host-side pad + TF32-round + relayout into per-core in_maps."""
    s_prev = np.asarray(s_prev, dtype=np.float32)
    h_j = np.asarray(h_j, dtype=np.float32)
    W_a = np.asarray(W_a, dtype=np.float32)
    U_a = np.asarray(U_a, dtype=np.float32)
    v_a = np.asarray(v_a, dtype=np.float32)

    gemm_np = np.float32 if GEMM_DT == F32R else mybir.dt.np(GEMM_DT)

    # U^T: [c, h] -> [k, p, h] with c = k*128 + p
    upad = np.zeros((HP, CP), np.float32)
    upad[:H, :C] = _tf32(U_a) if GEMM_DT == F32R else U_a
    u_prep = np.ascontiguousarray(upad.T.reshape(KC, 128, HP).astype(gemm_np))

    # bias ws[h, b] = sum_d W_a[h, d] s_prev[b, d], laid out [p, hk*BL + b]
    ws = s_prev @ W_a.T  # [B, H] fp32
    wpad = np.zeros((B, HP), np.float32)
    wpad[:, :H] = ws

    # v: [h] -> [p, hk]
    vpad = np.zeros((HP,), np.float32)
    vpad[:H] = v_a
    v_prep = np.ascontiguousarray(vpad.reshape(KH, 128).T)
    vg_prep = np.ascontiguousarray(
        np.repeat(v_prep[:, :, None], 128, axis=2).reshape(128, KH * 128)
    ).astype(gemm_np)
    one_prep = np.ones((128, 128), np.float32)

    hjr = _tf32(h_j) if GEMM_DT == F32R else h_j  # (B, NSEQ, C)

    in_maps = []
    for i in range(NCORES):
        x = hjr[i * BL : (i + 1) * BL].reshape(ML, C)
        xpad = np.zeros((CP, ML), gemm_np)
        xpad[:C, :] = x.T.astype(gemm_np)
        # [c, m] -> [t, p, k, j]: c = k*128+p, m = t*512+j
        hj_prep = np.ascontiguousarray(
            xpad.reshape(KC, 128, T, MT).transpose(2, 1, 0, 3)
        ).reshape(T, 128, KC * MT)
        wsl = wpad[i * BL : (i + 1) * BL]  # [BL, HP]
        ws_prep = np.ascontiguousarray(
            wsl.T.reshape(KH, 128, BL).transpose(1, 0, 2)
        ).reshape(128, KH * BL)
        in_maps.append(
            {
                "hj": hj_prep,
                "u": u_prep,
                "ws": ws_prep,
                "v": v_prep,
                "vg": vg_prep,
                "one": one_prep,
            }
        )
    return in_maps


def run(s_prev, h_j, W_a, U_a, v_a, trace=False, **trace_kwargs):
    """Run on 8 cores; returns (scores, BassKernelResults)."""
    nc = _get_nc()
    in_maps = _prep_inputs(s_prev, h_j, W_a, U_a, v_a)
    res = run_bass_kernel_spmd(
        nc, in_maps, core_ids=list(range(NCORES)), trace=trace, **trace_kwargs
    )
    scores = np.concatenate(
        [res.results[i]["out"] for i in range(NCORES)], axis=0
    ).astype(np.float32)
    return scores, res


def kernel(s_prev, h_j, W_a, U_a, v_a):
    scores, _ = run(s_prev, h_j, W_a, U_a, v_a, trace=False)
    return scores
